# revision 11
# baseline (speedup 1.0000x reference)
"""Trainium2 Bass kernel for ConvolutionalAttention (B=2,S=2048,E=1024,H=16,KS=3).

Reference:  Q,K,V = query @ W.T + b;  scores = QK^T/sqrt(Dh) per head;
cross-head conv1d (H->H channels, kernel 3) along the key axis; softmax over
keys; out = (weights @ V) merged heads @ Wo.T + bo.

Strategy (8 cores, head-parallel, conv folded into K, fp8 QK):
  K_conv[ho][k,(hi,d)] = sum_dk conv_w[ho,hi,dk] * K[k+dk-1,(hi,d)]
  => scores_conv[ho] = Q_full @ K_conv[ho]^T   (E=1024-deep matmul, computed
  transposed as [k,q]).  Each core owns H/8 = 2 output heads for all (b,q):
    1. each core computes only its OWN s-chunk (512 cols) of Q^T and K^T
       (stationary packed weight stripes), evacuates Q^T as fp8e4 and K^T
       as bf16 to DRAM, then AllGathers both across the 8 cores; V for the
       core's 2 heads is computed locally from query^T halves (the V
       matmuls fill the TensorE idle while the AllGathers fly);
    2. K_conv is formed on VectorE (tensor_scalar + 2 fused
       scalar_tensor_tensor AXPYs, bf16) + a ScalarE fp8 downcast into a
       RESIDENT fp8 SBUF tensor [128, HPC, NE, BS], in four (head, batch)
       passes ordered to unblock the first attention block earliest;
       Q^T fp8 is loaded into a resident [128, NE, BS] SBUF tensor.
       No DRAM traffic at all inside the attention loops.
    3. per (b, head): QK_conv as fp8 DoubleRow matmuls (256-deep per pass,
       [128,2,128] stationaries x [128,2,512] moving, 4 q-chunks per
       stationary) -> PSUM f32 -> Exp on ScalarE (bf16 out) -> PV matmuls
       against ones-augmented V so the softmax denominator lands in PSUM
       row 64 -> reciprocal -> K=1-matmul broadcast -> normalize (+bv,
       bf16 out).  bv is exact post-softmax (weights sum to 1); conv_b
       cancels inside softmax; 1/sqrt(Dh) folded into Wq/bq on host;
    4. AllToAll (bf16) reshards (head-slice -> q-slice); final Wo
       projection of this core's 512 output rows (bf16, f32 accum + bias).
"""
import numpy as np
import ml_dtypes

import concourse.bacc as bacc
import concourse.mybir as mybir
import concourse.tile as tile
from concourse.bass_utils import run_bass_kernel_spmd

B, S, E, H, KS = 2, 2048, 1024, 16, 3
DH = E // H                  # 64
N_CORES = 8
HPC = H // N_CORES           # 2 heads per core
BS = B * S                   # 4096
QSLICE = BS // N_CORES       # 512 output rows per core
NE = E // 128                # 8 contraction chunks
NEP = NE // 2                # 4 fp8 DoubleRow e-chunk pairs
NKT = S // 128               # 16 k-tiles per batch
NQC = S // 512               # 4 q-chunks per batch
VROW = DH + 1                # 65: head block in augmented V
KTC_W = S + 2                # per-batch K^T chunk: [z | b:S | z]

F32 = mybir.dt.float32
BF16 = mybir.dt.bfloat16
F8 = mybir.dt.float8e4
AL = mybir.AluOpType
AF = mybir.ActivationFunctionType
DR = mybir.MatmulPerfMode.DoubleRow


def build_nc(n_cores=N_CORES, collective=True):
    nc = bacc.Bacc("TRN2", target_bir_lowering=False, debug=False,
                   num_devices=n_cores)
    # inputs (host-prepped layouts; see prep_in_maps)
    qTh = nc.dram_tensor("qTh", [E, BS], BF16, kind="ExternalInput")
    wq_p = nc.dram_tensor("wq_p", [128, NE * NE * 128], BF16, kind="ExternalInput")
    wk_p = nc.dram_tensor("wk_p", [128, NE * NE * 128], BF16, kind="ExternalInput")
    wv_p = nc.dram_tensor("wv_p", [128, NE * HPC * DH], BF16, kind="ExternalInput")
    wo_p = nc.dram_tensor("wo_p", [128, NE * E], BF16, kind="ExternalInput")
    bq = nc.dram_tensor("bq", [128, NE], F32, kind="ExternalInput")
    bk = nc.dram_tensor("bk", [128, NE], F32, kind="ExternalInput")
    bv = nc.dram_tensor("bv", [128, HPC], F32, kind="ExternalInput")
    bo = nc.dram_tensor("bo", [128, E], F32, kind="ExternalInput")
    wvec = nc.dram_tensor("wvec", [128, HPC * KS * NE], F32, kind="ExternalInput")
    out = nc.dram_tensor("out", [QSLICE, E], F32, kind="ExternalOutput")
    # own s-chunk of query^T, supplied per-core (core c gets cols c*512..)
    qSl = nc.dram_tensor("qSl", [E, QSLICE], BF16, kind="ExternalInput")

    with tile.TileContext(nc) as tc:
        with (
            tc.tile_pool(name="dram", bufs=1, space="DRAM") as dram,
            tc.tile_pool(name="persist", bufs=1) as persist,
        ):
            k_own = dram.tile([E, QSLICE], BF16)
            q_own = dram.tile([E, QSLICE], F8)
            k_all = dram.tile([N_CORES * E, QSLICE], BF16, addr_space="Shared")
            q_all = dram.tile([N_CORES * E, QSLICE], F8, addr_space="Shared")
            a2a_in = dram.tile([N_CORES * 128, QSLICE], BF16)
            a2a_out = dram.tile([N_CORES * 128, QSLICE], BF16)

            # resident attention operands
            kc_sb = persist.tile([128, HPC, NE, BS], F8)   # K_conv (fp8)
            q_sb = persist.tile([128, NE, BS], F8)         # Q^T (fp8)
            # augmented V: cols = g*(HPC*VROW) + h*VROW + [0..63]=d, 64=ones
            # where g = b*NKT + kt is the global k-tile index (32 of them)
            v_sb = persist.tile([128, B * NKT * HPC * VROW], BF16)
            bv_sb = persist.tile([128, HPC], F32)
            wvec_sb = persist.tile([128, HPC * KS * NE], F32)
            ones_sb = persist.tile([1, DH], BF16)
            wo_sb = persist.tile([128, NE * E], BF16)
            bo_sb = persist.tile([128, E], F32)
            nc.sync.dma_start(bv_sb[:], bv[:, :])
            nc.sync.dma_start(wvec_sb[:], wvec[:, :])
            nc.vector.memset(ones_sb[:], 1.0)
            for g in range(B * NKT):
                for h in range(HPC):
                    c0 = g * HPC * VROW + h * VROW + DH
                    nc.vector.memset(v_sb[:, c0:c0 + 1], 1.0)

            # -------- phase 1a: own-chunk Q/K projections + AllGathers -----
            with (
                tc.tile_pool(name="proj", bufs=1) as proj,
                tc.tile_pool(name="pw", bufs=2) as pw,
                tc.tile_pool(name="pevac", bufs=3) as pevac,
                tc.tile_pool(name="ppsum", bufs=3, space="PSUM") as ppsum,
            ):
                qt_own = proj.tile([128, NE * QSLICE], BF16, tag="qtown")
                bq_sb = proj.tile([128, NE], F32, tag="bq")
                bk_sb = proj.tile([128, NE], F32, tag="bk")

                # own s-chunk of raw query^T first: it + the first K stripe
                # gate the K projection whose AllGather is the critical path
                for j in range(NE):
                    nc.sync.dma_start(
                        qt_own[:, j * QSLICE:(j + 1) * QSLICE],
                        qSl[j * 128:(j + 1) * 128, :])
                # prefetch first two weight stripes
                wqk_pre = []
                for et in range(2):
                    wq_sb = pw.tile([128, NE * 128], BF16, tag="wqs",
                                    name=f"wqp{et}")
                    wk_sb = pw.tile([128, NE * 128], BF16, tag="wks",
                                    name=f"wkp{et}")
                    nc.sync.dma_start(wk_sb[:], wk_p[:, et * E:(et + 1) * E])
                    nc.sync.dma_start(wq_sb[:], wq_p[:, et * E:(et + 1) * E])
                    wqk_pre.append((wq_sb, wk_sb))
                nc.sync.dma_start(bk_sb[:], bk[:, :])
                nc.sync.dma_start(bq_sb[:], bq[:, :])

                # K^T own chunk -> bf16 DRAM; Q^T own chunk -> fp8 DRAM
                for which in ("k", "q"):
                    for et in range(NE):
                        if et < 2:
                            w_sb = wqk_pre[et][0 if which == "q" else 1]
                        else:
                            w_sb = pw.tile([128, NE * 128], BF16,
                                           tag=f"w{which}s")
                            src = wq_p if which == "q" else wk_p
                            nc.sync.dma_start(
                                w_sb[:], src[:, et * E:(et + 1) * E])
                        ps = ppsum.tile([128, QSLICE], F32, tag="pp")
                        for j in range(NE):
                            nc.tensor.matmul(
                                ps[:], w_sb[:, j * 128:(j + 1) * 128],
                                qt_own[:, j * QSLICE:(j + 1) * QSLICE],
                                start=(j == 0), stop=(j == NE - 1))
                        if which == "k":
                            ke = pevac.tile([128, QSLICE], BF16, tag="kevac")
                            nc.scalar.activation(ke[:], ps[:], AF.Identity,
                                                 bias=bk_sb[:, et:et + 1],
                                                 scale=1.0)
                            nc.sync.dma_start(
                                k_own[et * 128:(et + 1) * 128, :], ke[:])
                        else:
                            qe = pevac.tile([128, QSLICE], F8, tag="qevac")
                            nc.scalar.activation(qe[:], ps[:], AF.Identity,
                                                 bias=bq_sb[:, et:et + 1],
                                                 scale=1.0)
                            nc.sync.dma_start(
                                q_own[et * 128:(et + 1) * 128, :], qe[:])
                    if which == "k" and collective:
                        nc.gpsimd.collective_compute(
                            "AllGather", AL.bypass,
                            replica_groups=[list(range(n_cores))],
                            ins=[k_own.opt()], outs=[k_all.opt()])
                if collective:
                    nc.gpsimd.collective_compute(
                        "AllGather", AL.bypass,
                        replica_groups=[list(range(n_cores))],
                        ins=[q_own.opt()], outs=[q_all.opt()])

            # -------- phase 1b: V (own 2 heads) from query^T halves --------
            # fills the TensorE idle while the AllGathers run on TOPSP/SDMA
            with (
                tc.tile_pool(name="vproj", bufs=1) as vproj,
                tc.tile_pool(name="qth", bufs=2) as qth,
                tc.tile_pool(name="vpsum", bufs=2, space="PSUM") as vpsum,
            ):
                wv_sb = vproj.tile([128, NE * HPC * DH], BF16, tag="wv")
                nc.sync.dma_start(wv_sb[:], wv_p[:, :])
                for b_i in range(B):
                    qh = qth.tile([128, NE * S], BF16, tag="qth")
                    for j in range(NE):
                        nc.sync.dma_start(
                            qh[:, j * S:(j + 1) * S],
                            qTh[j * 128:(j + 1) * 128,
                                b_i * S:(b_i + 1) * S])
                    for gl in range(NKT):          # local s-tile within batch
                        g = b_i * NKT + gl
                        pv = vpsum.tile([128, HPC * DH], F32, tag="pv")
                        for j in range(NE):
                            nc.tensor.matmul(
                                pv[:], qh[:, j * S + gl * 128:
                                          j * S + (gl + 1) * 128],
                                wv_sb[:, j * HPC * DH:(j + 1) * HPC * DH],
                                start=(j == 0), stop=(j == NE - 1))
                        for h in range(HPC):
                            c0 = g * HPC * VROW + h * VROW
                            nc.scalar.activation(
                                v_sb[:, c0:c0 + DH],
                                pv[:, h * DH:(h + 1) * DH], AF.Copy)

            # -------- phase 2: form K_conv (fp8) + load Q^T fp8 --------
            with (
                tc.tile_pool(name="ktp", bufs=3) as ktp,
                tc.tile_pool(name="kcv", bufs=2) as kcv,
            ):
                # four (head, batch) passes, first-attention-block first;
                # each pass streams the 8 e-chunks of K^T for its batch
                for pi_, (h, b_i) in enumerate(((0, 0), (1, 0), (0, 1), (1, 1))):
                    for cc in range(NE):
                        ktc = ktp.tile([128, KTC_W], BF16, tag="ktc")
                        nc.vector.memset(ktc[:, 0:1], 0.0)
                        nc.vector.memset(ktc[:, S + 1:S + 2], 0.0)
                        for pos in range(NQC):
                            sc = b_i * NQC + pos   # source core for this span
                            nc.sync.dma_start(
                                ktc[:, 1 + pos * QSLICE:1 + (pos + 1) * QSLICE],
                                k_all[sc * E + cc * 128:
                                      sc * E + (cc + 1) * 128, :])

                        def wv_(dk):
                            col = (h * KS + dk) * NE + cc
                            return wvec_sb[:, col:col + 1]

                        # all on VectorE: ScalarE stays free for the EXPs
                        # (its FIFO queue would otherwise head-of-line block
                        # attention behind the downcasts)
                        t0 = kcv.tile([128, S], BF16, tag="t0")
                        t1 = kcv.tile([128, S], BF16, tag="t1")
                        nc.vector.tensor_scalar(
                            t0[:], ktc[:, 0:S], wv_(0), None, AL.mult)
                        nc.vector.tensor_scalar(
                            t1[:], ktc[:, 1:1 + S], wv_(1), None, AL.mult)
                        nc.vector.tensor_tensor(t0[:], t0[:], t1[:], AL.add)
                        nc.vector.tensor_scalar(
                            t1[:], ktc[:, 2:2 + S], wv_(2), None, AL.mult)
                        nc.vector.tensor_tensor(
                            kc_sb[:, h, cc, b_i * S:(b_i + 1) * S],
                            t0[:], t1[:], AL.add)
                    if pi_ == 1:
                        # Q^T fp8 resident load, after the first two K_conv
                        # passes' loads (readiness order: AG_K before AG_Q)
                        for j in range(NE):
                            for sc in range(N_CORES):
                                nc.sync.dma_start(
                                    q_sb[:, j, sc * QSLICE:(sc + 1) * QSLICE],
                                    q_all[sc * E + j * 128:
                                          sc * E + (j + 1) * 128, :])

            # ---------------- phase 3: attention ----------------
            with (
                tc.tile_pool(name="esb", bufs=6) as esb,
                tc.tile_pool(name="norm", bufs=4) as norm,
                tc.tile_pool(name="qkpsum", bufs=4, space="PSUM") as qkpsum,
                tc.tile_pool(name="pvpsum", bufs=4, space="PSUM") as pvpsum,
            ):
                for b_i in range(B):
                    for h in range(HPC):
                        pvs = [pvpsum.tile([VROW, 512], F32, tag="pvp",
                                           name=f"pv{qq}")
                               for qq in range(NQC)]
                        inv_sb = norm.tile([1, S], BF16, tag="inv")
                        for kt in range(NKT):
                            g = b_i * NKT + kt
                            c0 = g * HPC * VROW + h * VROW
                            pss = [qkpsum.tile([128, 512], F32, tag="qk",
                                               name=f"qk{qc}")
                                   for qc in range(NQC)]
                            for c in range(NEP):
                                lhsT = kc_sb[:, h, 2 * c:2 * c + 2,
                                             b_i * S + kt * 128:
                                             b_i * S + (kt + 1) * 128]
                                for qc in range(NQC):
                                    rhs = q_sb[:, 2 * c:2 * c + 2,
                                               b_i * S + qc * 512:
                                               b_i * S + (qc + 1) * 512]
                                    nc.tensor.matmul(
                                        pss[qc][:], lhsT, rhs,
                                        start=(c == 0), stop=(c == NEP - 1),
                                        perf_mode=DR)
                            for qc in range(NQC):
                                ex = esb.tile([128, 512], BF16, tag="exp")
                                nc.scalar.activation(ex[:], pss[qc][:], AF.Exp)
                                nc.tensor.matmul(
                                    pvs[qc][:], v_sb[:, c0:c0 + VROW],
                                    ex[:], start=(kt == 0),
                                    stop=(kt == NKT - 1))
                        # normalize + bias, ship to a2a bounce
                        for qc in range(NQC):
                            with nc.allow_low_precision(
                                    reason="softmax denom bf16 bcast"):
                                nc.vector.reciprocal(
                                    inv_sb[0:1, qc * 512:(qc + 1) * 512],
                                    pvs[qc][DH:DH + 1, :])
                            # shares the qk bank ring (4 qk + 4 pv = 8 banks)
                            pi = qkpsum.tile([DH, 512], F32, tag="qk",
                                             name="pi")
                            nc.tensor.matmul(
                                pi[:], ones_sb[0:1, :],
                                inv_sb[0:1, qc * 512:(qc + 1) * 512],
                                start=True, stop=True)
                            ib = norm.tile([DH, 512], F32, tag="invbc")
                            nc.scalar.activation(ib[:], pi[:], AF.Copy)
                            ho = norm.tile([DH, 512], BF16, tag="ho")
                            nc.vector.tensor_tensor(ho[:], pvs[qc][0:DH, :],
                                                    ib[:], AL.mult)
                            nc.vector.tensor_scalar(
                                ho[:], ho[:], bv_sb[0:DH, h:h + 1], None, AL.add)
                            piece = b_i * NQC + qc
                            r0 = piece * 128 + h * DH
                            nc.sync.dma_start(a2a_in[r0:r0 + DH, :], ho[:])

            # output-projection weights: loaded late so the bulk DMAs do
            # not delay the phase-1 critical path
            nc.sync.dma_start(wo_sb[:], wo_p[:, :])
            nc.sync.dma_start(bo_sb[:], bo[:, :])

            # ---------------- phase 4: exchange + output proj ----------------
            if collective:
                nc.gpsimd.collective_compute(
                    "AllToAll", AL.bypass,
                    replica_groups=[list(range(n_cores))],
                    ins=[a2a_in.opt()], outs=[a2a_out.opt()])
            else:
                nc.sync.dma_start(a2a_out[:, :], a2a_in[:, :])

            with (
                tc.tile_pool(name="fin", bufs=1) as fin,
                tc.tile_pool(name="fevac", bufs=3) as fevac,
                tc.tile_pool(name="fpsum", bufs=2, space="PSUM") as fpsum,
            ):
                go_sb = fin.tile([128, NE * QSLICE], BF16, tag="go")
                for e in range(NE):
                    nc.sync.dma_start(go_sb[:, e * QSLICE:(e + 1) * QSLICE],
                                      a2a_out[e * 128:(e + 1) * 128, :])
                NH = E // 512
                for qt in range(QSLICE // 128):
                    pfs = [fpsum.tile([128, 512], F32, tag="pf",
                                      name=f"pf{nh}") for nh in range(NH)]
                    for e in range(NE):
                        lhsT = go_sb[:, e * QSLICE + qt * 128:
                                     e * QSLICE + (qt + 1) * 128]
                        for nh in range(NH):
                            rhs = wo_sb[:, e * E + nh * 512:
                                        e * E + (nh + 1) * 512]
                            nc.tensor.matmul(pfs[nh][:], lhsT, rhs,
                                             start=(e == 0),
                                             stop=(e == NE - 1))
                    for nh in range(NH):
                        ot = fevac.tile([128, 512], F32, tag="ot")
                        nc.vector.tensor_tensor(
                            ot[:], pfs[nh][:], bo_sb[:, nh * 512:(nh + 1) * 512],
                            AL.add)
                        nc.sync.dma_start(
                            out[qt * 128:(qt + 1) * 128,
                                nh * 512:(nh + 1) * 512], ot[:])
    nc.compile()
    return nc


def prep_in_maps(query, Wq, bq, Wk, bk, Wv, bv, Wo, bo, conv_w, conv_b):
    """Host-side layout prep. conv_b is dropped: softmax(x+c) == softmax(x)."""
    del conv_b
    scale = 1.0 / np.sqrt(DH)
    qT = np.ascontiguousarray(query.reshape(BS, E).T)           # [E, BS]
    qTh = qT.astype(ml_dtypes.bfloat16)

    def pack_w(WT):  # [E_in, E_out] -> [128, NE*NE*128], stripe et is
        # [128, NE*128] with col (j*128+e) = WT[j*128+p, et*128+e]
        Wr = WT.reshape(NE, 128, NE, 128)          # [j, p, et, e]
        return np.ascontiguousarray(
            Wr.transpose(1, 2, 0, 3).reshape(128, NE * NE * 128))

    wq_p = pack_w((Wq.T * scale).astype(np.float32)).astype(ml_dtypes.bfloat16)
    wk_p = pack_w(Wk.T.astype(np.float32)).astype(ml_dtypes.bfloat16)
    # wo packed: stripe e is [128, E] with col eo = Wo.T[e*128+p, eo]
    wo_p = np.ascontiguousarray(
        Wo.T.reshape(NE, 128, E).transpose(1, 0, 2).reshape(128, NE * E)
    ).astype(ml_dtypes.bfloat16)
    bq_a = np.ascontiguousarray((bq * scale).reshape(NE, 128).T).astype(np.float32)
    bk_a = np.ascontiguousarray(bk.reshape(NE, 128).T).astype(np.float32)
    bo_a = np.tile(bo.astype(np.float32)[None, :], (128, 1))

    in_maps = []
    for c in range(N_CORES):
        heads = [HPC * c + h for h in range(HPC)]
        # Wv^T slice packed: [128, NE*HPC*DH], col block j -> Wv[e_g, j*128+p]
        wv_cols = np.concatenate(
            [Wv[ho * DH:(ho + 1) * DH, :] for ho in heads], axis=0)  # [128,E]
        wv_p = np.ascontiguousarray(
            wv_cols.T.reshape(NE, 128, HPC * DH).transpose(1, 0, 2)
            .reshape(128, NE * HPC * DH)).astype(ml_dtypes.bfloat16)
        bv_a = np.zeros((128, HPC), np.float32)
        for h, ho in enumerate(heads):
            bv_a[0:DH, h] = bv[ho * DH:(ho + 1) * DH]
        wvec = np.zeros((128, HPC * KS * NE), np.float32)
        for h, ho in enumerate(heads):
            for dk in range(KS):
                col_v = np.repeat(conv_w[ho, :, dk], DH)       # [E]
                for cc in range(NE):
                    wvec[:, (h * KS + dk) * NE + cc] = col_v[cc * 128:(cc + 1) * 128]
        in_maps.append({
            "qTh": qTh,
            "qSl": np.ascontiguousarray(qTh[:, c * QSLICE:(c + 1) * QSLICE]),
            "wq_p": wq_p, "wk_p": wk_p, "wv_p": wv_p,
            "wo_p": wo_p, "bq": bq_a, "bk": bk_a, "bv": bv_a,
            "bo": bo_a, "wvec": wvec,
        })
    return in_maps


_NC_CACHE = {}


def kernel(**inputs) -> np.ndarray:
    in_maps = prep_in_maps(**{k: np.asarray(v) for k, v in inputs.items()})
    if "nc" not in _NC_CACHE:
        _NC_CACHE["nc"] = build_nc()
    nc = _NC_CACHE["nc"]
    res = run_bass_kernel_spmd(nc, in_maps, list(range(N_CORES)))
    full = np.concatenate([res.results[c]["out"] for c in range(N_CORES)],
                          axis=0)
    return full.reshape(B, S, E).astype(np.float32)


# revision 14
# speedup vs baseline: 1.0546x; 1.0546x over previous
"""Trainium2 Bass kernel for ConvolutionalAttention (B=2,S=2048,E=1024,H=16,KS=3).

Reference:  Q,K,V = query @ W.T + b;  scores = QK^T/sqrt(Dh) per head;
cross-head conv1d (H->H channels, kernel 3) along the key axis; softmax over
keys; out = (weights @ V) merged heads @ Wo.T + bo.

Strategy (8 cores, head-parallel, conv folded into K, fp8 QK):
  K_conv[ho][k,(hi,d)] = sum_dk conv_w[ho,hi,dk] * K[k+dk-1,(hi,d)]
  => scores_conv[ho] = Q_full @ K_conv[ho]^T   (E=1024-deep matmul, computed
  transposed as [k,q]).  Each core owns H/8 = 2 output heads for all (b,q):
    1. each core computes only its OWN s-chunk (512 cols) of Q^T and K^T
       (stationary packed weight stripes), evacuates Q^T as fp8e4 and K^T
       as bf16 to DRAM, then AllGathers both across the 8 cores; V for the
       core's 2 heads is computed locally from query^T halves (the V
       matmuls fill the TensorE idle while the AllGathers fly);
    2. K_conv is formed on VectorE (tensor_scalar + 2 fused
       scalar_tensor_tensor AXPYs, bf16) + a ScalarE fp8 downcast into a
       RESIDENT fp8 SBUF tensor [128, HPC, NE, BS], in four (head, batch)
       passes ordered to unblock the first attention block earliest;
       Q^T fp8 is loaded into a resident [128, NE, BS] SBUF tensor.
       No DRAM traffic at all inside the attention loops.
    3. per (b, head): QK_conv as fp8 DoubleRow matmuls (256-deep per pass,
       [128,2,128] stationaries x [128,2,512] moving, 4 q-chunks per
       stationary) -> PSUM f32 -> Exp on ScalarE (bf16 out) -> PV matmuls
       against ones-augmented V so the softmax denominator lands in PSUM
       row 64 -> reciprocal -> K=1-matmul broadcast -> normalize (+bv,
       bf16 out).  bv is exact post-softmax (weights sum to 1); conv_b
       cancels inside softmax; 1/sqrt(Dh) folded into Wq/bq on host;
    4. AllToAll (bf16) reshards (head-slice -> q-slice); final Wo
       projection of this core's 512 output rows (bf16, f32 accum + bias).
"""
import numpy as np
import ml_dtypes

import concourse.bacc as bacc
import concourse.mybir as mybir
import concourse.tile as tile
from concourse.bass_utils import run_bass_kernel_spmd

B, S, E, H, KS = 2, 2048, 1024, 16, 3
DH = E // H                  # 64
N_CORES = 8
HPC = H // N_CORES           # 2 heads per core
BS = B * S                   # 4096
QSLICE = BS // N_CORES       # 512 output rows per core
NE = E // 128                # 8 contraction chunks
NEP = NE // 2                # 4 fp8 DoubleRow e-chunk pairs
NKT = S // 128               # 16 k-tiles per batch
NQC = S // 512               # 4 q-chunks per batch
VROW = DH + 1                # 65: head block in augmented V
KTC_W = S + 2                # per-batch K^T chunk: [z | b:S | z]

F32 = mybir.dt.float32
BF16 = mybir.dt.bfloat16
F8 = mybir.dt.float8e4
AL = mybir.AluOpType
AF = mybir.ActivationFunctionType
DR = mybir.MatmulPerfMode.DoubleRow


def build_nc(n_cores=N_CORES, collective=True):
    nc = bacc.Bacc("TRN2", target_bir_lowering=False, debug=False,
                   num_devices=n_cores)
    # inputs (host-prepped layouts; see prep_in_maps)
    qTh = nc.dram_tensor("qTh", [E, BS], BF16, kind="ExternalInput")
    wq_p = nc.dram_tensor("wq_p", [128, NE * NE * 128], BF16, kind="ExternalInput")
    wk_p = nc.dram_tensor("wk_p", [128, NE * NE * 128], BF16, kind="ExternalInput")
    wv_p = nc.dram_tensor("wv_p", [128, NE * HPC * DH], BF16, kind="ExternalInput")
    wo_p = nc.dram_tensor("wo_p", [128, NE * E], BF16, kind="ExternalInput")
    bq = nc.dram_tensor("bq", [128, NE], F32, kind="ExternalInput")
    bk = nc.dram_tensor("bk", [128, NE], F32, kind="ExternalInput")
    bv = nc.dram_tensor("bv", [128, HPC], F32, kind="ExternalInput")
    bo = nc.dram_tensor("bo", [128, E], F32, kind="ExternalInput")
    wvec = nc.dram_tensor("wvec", [128, HPC * KS * NE], F32, kind="ExternalInput")
    out = nc.dram_tensor("out", [QSLICE, E], F32, kind="ExternalOutput")
    # own s-chunk of query^T, supplied per-core (core c gets cols c*512..)
    qSl = nc.dram_tensor("qSl", [E, QSLICE], BF16, kind="ExternalInput")

    with tile.TileContext(nc) as tc:
        with (
            tc.tile_pool(name="dram", bufs=1, space="DRAM") as dram,
            tc.tile_pool(name="persist", bufs=1) as persist,
        ):
            k_own = dram.tile([E, QSLICE], BF16)
            q_own = dram.tile([E, QSLICE], F8)
            k_all = dram.tile([N_CORES * E, QSLICE], BF16, addr_space="Shared")
            q_all = dram.tile([N_CORES * E, QSLICE], F8, addr_space="Shared")
            a2a_in = dram.tile([N_CORES * 128, QSLICE], BF16)
            a2a_out = dram.tile([N_CORES * 128, QSLICE], BF16)

            # resident attention operands
            kc_sb = persist.tile([128, HPC, NE, BS], F8)   # K_conv (fp8)
            q_sb = persist.tile([128, NE, BS], F8)         # Q^T (fp8)
            # augmented V: cols = g*(HPC*VROW) + h*VROW + [0..63]=d, 64=ones
            # where g = b*NKT + kt is the global k-tile index (32 of them)
            v_sb = persist.tile([128, B * NKT * HPC * VROW], BF16)
            bv_sb = persist.tile([128, HPC], F32)
            wvec_sb = persist.tile([128, HPC * KS * NE], F32)
            ones_sb = persist.tile([1, DH], BF16)
            wo_sb = persist.tile([128, NE * E], BF16)
            bo_sb = persist.tile([128, E], F32)
            nc.sync.dma_start(bv_sb[:], bv[:, :])
            nc.sync.dma_start(wvec_sb[:], wvec[:, :])
            nc.vector.memset(ones_sb[:], 1.0)
            for g in range(B * NKT):
                for h in range(HPC):
                    c0 = g * HPC * VROW + h * VROW + DH
                    nc.vector.memset(v_sb[:, c0:c0 + 1], 1.0)

            # -------- phase 1a: own-chunk Q/K projections + AllGathers -----
            with (
                tc.tile_pool(name="proj", bufs=1) as proj,
                tc.tile_pool(name="pw", bufs=NE) as pw,
                tc.tile_pool(name="pevac", bufs=3) as pevac,
                tc.tile_pool(name="ppsum", bufs=3, space="PSUM") as ppsum,
            ):
                qt_own = proj.tile([128, NE * QSLICE], BF16, tag="qtown")
                bq_sb = proj.tile([128, NE], F32, tag="bq")
                bk_sb = proj.tile([128, NE], F32, tag="bk")

                # own s-chunk of raw query^T first: it + the first K stripe
                # gate the K projection whose AllGather is the critical path
                for j in range(NE):
                    nc.sync.dma_start(
                        qt_own[:, j * QSLICE:(j + 1) * QSLICE],
                        qSl[j * 128:(j + 1) * 128, :])
                # prefetch ALL weight stripes upfront (K first): the K-chunk
                # evac writes must not queue behind trickling stripe DMAs,
                # since the K AllGather trigger is the critical path
                wk_stripes, wq_stripes = [], []
                for et in range(NE):
                    wk_sb = pw.tile([128, NE * 128], BF16, tag="wks",
                                    name=f"wkp{et}")
                    nc.sync.dma_start(wk_sb[:], wk_p[:, et * E:(et + 1) * E])
                    wk_stripes.append(wk_sb)
                nc.sync.dma_start(bk_sb[:], bk[:, :])
                for et in range(NE):
                    wq_sb = pw.tile([128, NE * 128], BF16, tag="wqs",
                                    name=f"wqp{et}")
                    nc.sync.dma_start(wq_sb[:], wq_p[:, et * E:(et + 1) * E])
                    wq_stripes.append(wq_sb)
                nc.sync.dma_start(bq_sb[:], bq[:, :])

                # K^T own chunk -> bf16 DRAM; Q^T own chunk -> fp8 DRAM
                for which in ("k", "q"):
                    for et in range(NE):
                        w_sb = (wq_stripes if which == "q" else wk_stripes)[et]
                        ps = ppsum.tile([128, QSLICE], F32, tag="pp")
                        for j in range(NE):
                            nc.tensor.matmul(
                                ps[:], w_sb[:, j * 128:(j + 1) * 128],
                                qt_own[:, j * QSLICE:(j + 1) * QSLICE],
                                start=(j == 0), stop=(j == NE - 1))
                        if which == "k":
                            ke = pevac.tile([128, QSLICE], BF16, tag="kevac")
                            nc.scalar.activation(ke[:], ps[:], AF.Identity,
                                                 bias=bk_sb[:, et:et + 1],
                                                 scale=1.0)
                            nc.sync.dma_start(
                                k_own[et * 128:(et + 1) * 128, :], ke[:])
                        else:
                            qe = pevac.tile([128, QSLICE], F8, tag="qevac")
                            nc.scalar.activation(qe[:], ps[:], AF.Identity,
                                                 bias=bq_sb[:, et:et + 1],
                                                 scale=1.0)
                            nc.sync.dma_start(
                                q_own[et * 128:(et + 1) * 128, :], qe[:])
                    if which == "k" and collective:
                        nc.gpsimd.collective_compute(
                            "AllGather", AL.bypass,
                            replica_groups=[list(range(n_cores))],
                            ins=[k_own.opt()], outs=[k_all.opt()])
                if collective:
                    nc.gpsimd.collective_compute(
                        "AllGather", AL.bypass,
                        replica_groups=[list(range(n_cores))],
                        ins=[q_own.opt()], outs=[q_all.opt()])

            # -------- phase 1b: V (own 2 heads) from query^T halves --------
            # fills the TensorE idle while the AllGathers run on TOPSP/SDMA
            with (
                tc.tile_pool(name="vproj", bufs=1) as vproj,
                tc.tile_pool(name="qth", bufs=2) as qth,
                tc.tile_pool(name="vpsum", bufs=2, space="PSUM") as vpsum,
            ):
                wv_sb = vproj.tile([128, NE * HPC * DH], BF16, tag="wv")
                nc.sync.dma_start(wv_sb[:], wv_p[:, :])
                for b_i in range(B):
                    qh = qth.tile([128, NE * S], BF16, tag="qth")
                    for j in range(NE):
                        nc.sync.dma_start(
                            qh[:, j * S:(j + 1) * S],
                            qTh[j * 128:(j + 1) * 128,
                                b_i * S:(b_i + 1) * S])
                    for gl in range(NKT):          # local s-tile within batch
                        g = b_i * NKT + gl
                        pv = vpsum.tile([128, HPC * DH], F32, tag="pv")
                        for j in range(NE):
                            nc.tensor.matmul(
                                pv[:], qh[:, j * S + gl * 128:
                                          j * S + (gl + 1) * 128],
                                wv_sb[:, j * HPC * DH:(j + 1) * HPC * DH],
                                start=(j == 0), stop=(j == NE - 1))
                        for h in range(HPC):
                            c0 = g * HPC * VROW + h * VROW
                            nc.scalar.activation(
                                v_sb[:, c0:c0 + DH],
                                pv[:, h * DH:(h + 1) * DH], AF.Copy)

            # -------- phase 2: form K_conv (fp8) + load Q^T fp8 --------
            with (
                tc.tile_pool(name="ktp", bufs=3) as ktp,
                tc.tile_pool(name="kcv", bufs=2) as kcv,
            ):
                # four (head, batch) passes, first-attention-block first;
                # each pass streams the 8 e-chunks of K^T for its batch
                for pi_, (h, b_i) in enumerate(((0, 0), (1, 0), (0, 1), (1, 1))):
                    for cc in range(NE):
                        ktc = ktp.tile([128, KTC_W], BF16, tag="ktc")
                        nc.vector.memset(ktc[:, 0:1], 0.0)
                        nc.vector.memset(ktc[:, S + 1:S + 2], 0.0)
                        for pos in range(NQC):
                            sc = b_i * NQC + pos   # source core for this span
                            nc.sync.dma_start(
                                ktc[:, 1 + pos * QSLICE:1 + (pos + 1) * QSLICE],
                                k_all[sc * E + cc * 128:
                                      sc * E + (cc + 1) * 128, :])

                        def wv_(dk):
                            col = (h * KS + dk) * NE + cc
                            return wvec_sb[:, col:col + 1]

                        # all on VectorE: ScalarE stays free for the EXPs
                        # (its FIFO queue would otherwise head-of-line block
                        # attention behind the downcasts)
                        t0 = kcv.tile([128, S], BF16, tag="t0")
                        t1 = kcv.tile([128, S], BF16, tag="t1")
                        nc.vector.tensor_scalar(
                            t0[:], ktc[:, 0:S], wv_(0), None, AL.mult)
                        nc.vector.tensor_scalar(
                            t1[:], ktc[:, 1:1 + S], wv_(1), None, AL.mult)
                        nc.vector.tensor_tensor(t0[:], t0[:], t1[:], AL.add)
                        nc.vector.tensor_scalar(
                            t1[:], ktc[:, 2:2 + S], wv_(2), None, AL.mult)
                        nc.vector.tensor_tensor(
                            kc_sb[:, h, cc, b_i * S:(b_i + 1) * S],
                            t0[:], t1[:], AL.add)
                    if pi_ == 1:
                        # Q^T fp8 resident load, after the first two K_conv
                        # passes' loads (readiness order: AG_K before AG_Q)
                        for j in range(NE):
                            for sc in range(N_CORES):
                                nc.sync.dma_start(
                                    q_sb[:, j, sc * QSLICE:(sc + 1) * QSLICE],
                                    q_all[sc * E + j * 128:
                                          sc * E + (j + 1) * 128, :])

            # ---------------- phase 3: attention ----------------
            with (
                tc.tile_pool(name="esb", bufs=6) as esb,
                tc.tile_pool(name="norm", bufs=4) as norm,
                tc.tile_pool(name="qkpsum", bufs=4, space="PSUM") as qkpsum,
                tc.tile_pool(name="pvpsum", bufs=4, space="PSUM") as pvpsum,
            ):
                for b_i in range(B):
                    for h in range(HPC):
                        pvs = [pvpsum.tile([VROW, 512], F32, tag="pvp",
                                           name=f"pv{qq}")
                               for qq in range(NQC)]
                        inv_sb = norm.tile([1, S], BF16, tag="inv")
                        for kt in range(NKT):
                            g = b_i * NKT + kt
                            c0 = g * HPC * VROW + h * VROW
                            pss = [qkpsum.tile([128, 512], F32, tag="qk",
                                               name=f"qk{qc}")
                                   for qc in range(NQC)]
                            for c in range(NEP):
                                lhsT = kc_sb[:, h, 2 * c:2 * c + 2,
                                             b_i * S + kt * 128:
                                             b_i * S + (kt + 1) * 128]
                                for qc in range(NQC):
                                    rhs = q_sb[:, 2 * c:2 * c + 2,
                                               b_i * S + qc * 512:
                                               b_i * S + (qc + 1) * 512]
                                    nc.tensor.matmul(
                                        pss[qc][:], lhsT, rhs,
                                        start=(c == 0), stop=(c == NEP - 1),
                                        perf_mode=DR)
                            for qc in range(NQC):
                                ex = esb.tile([128, 512], BF16, tag="exp")
                                nc.scalar.activation(ex[:], pss[qc][:], AF.Exp)
                                nc.tensor.matmul(
                                    pvs[qc][:], v_sb[:, c0:c0 + VROW],
                                    ex[:], start=(kt == 0),
                                    stop=(kt == NKT - 1))
                        # normalize + bias, ship to a2a bounce
                        for qc in range(NQC):
                            with nc.allow_low_precision(
                                    reason="softmax denom bf16 bcast"):
                                nc.vector.reciprocal(
                                    inv_sb[0:1, qc * 512:(qc + 1) * 512],
                                    pvs[qc][DH:DH + 1, :])
                            # shares the qk bank ring (4 qk + 4 pv = 8 banks)
                            pi = qkpsum.tile([DH, 512], F32, tag="qk",
                                             name="pi")
                            nc.tensor.matmul(
                                pi[:], ones_sb[0:1, :],
                                inv_sb[0:1, qc * 512:(qc + 1) * 512],
                                start=True, stop=True)
                            ib = norm.tile([DH, 512], F32, tag="invbc")
                            nc.scalar.activation(ib[:], pi[:], AF.Copy)
                            ho = norm.tile([DH, 512], BF16, tag="ho")
                            nc.vector.tensor_tensor(ho[:], pvs[qc][0:DH, :],
                                                    ib[:], AL.mult)
                            nc.vector.tensor_scalar(
                                ho[:], ho[:], bv_sb[0:DH, h:h + 1], None, AL.add)
                            piece = b_i * NQC + qc
                            r0 = piece * 128 + h * DH
                            nc.sync.dma_start(a2a_in[r0:r0 + DH, :], ho[:])

            # output-projection weights: loaded late so the bulk DMAs do
            # not delay the phase-1 critical path
            nc.sync.dma_start(wo_sb[:], wo_p[:, :])
            nc.sync.dma_start(bo_sb[:], bo[:, :])

            # ---------------- phase 4: exchange + output proj ----------------
            if collective:
                nc.gpsimd.collective_compute(
                    "AllToAll", AL.bypass,
                    replica_groups=[list(range(n_cores))],
                    ins=[a2a_in.opt()], outs=[a2a_out.opt()])
            else:
                nc.sync.dma_start(a2a_out[:, :], a2a_in[:, :])

            with (
                tc.tile_pool(name="fin", bufs=1) as fin,
                tc.tile_pool(name="fevac", bufs=3) as fevac,
                tc.tile_pool(name="fpsum", bufs=2, space="PSUM") as fpsum,
            ):
                go_sb = fin.tile([128, NE * QSLICE], BF16, tag="go")
                for e in range(NE):
                    nc.sync.dma_start(go_sb[:, e * QSLICE:(e + 1) * QSLICE],
                                      a2a_out[e * 128:(e + 1) * 128, :])
                NH = E // 512
                for qt in range(QSLICE // 128):
                    pfs = [fpsum.tile([128, 512], F32, tag="pf",
                                      name=f"pf{nh}") for nh in range(NH)]
                    for e in range(NE):
                        lhsT = go_sb[:, e * QSLICE + qt * 128:
                                     e * QSLICE + (qt + 1) * 128]
                        for nh in range(NH):
                            rhs = wo_sb[:, e * E + nh * 512:
                                        e * E + (nh + 1) * 512]
                            nc.tensor.matmul(pfs[nh][:], lhsT, rhs,
                                             start=(e == 0),
                                             stop=(e == NE - 1))
                    for nh in range(NH):
                        ot = fevac.tile([128, 512], F32, tag="ot")
                        nc.vector.tensor_tensor(
                            ot[:], pfs[nh][:], bo_sb[:, nh * 512:(nh + 1) * 512],
                            AL.add)
                        nc.sync.dma_start(
                            out[qt * 128:(qt + 1) * 128,
                                nh * 512:(nh + 1) * 512], ot[:])
    nc.compile()
    return nc


def prep_in_maps(query, Wq, bq, Wk, bk, Wv, bv, Wo, bo, conv_w, conv_b):
    """Host-side layout prep. conv_b is dropped: softmax(x+c) == softmax(x)."""
    del conv_b
    scale = 1.0 / np.sqrt(DH)
    qT = np.ascontiguousarray(query.reshape(BS, E).T)           # [E, BS]
    qTh = qT.astype(ml_dtypes.bfloat16)

    def pack_w(WT):  # [E_in, E_out] -> [128, NE*NE*128], stripe et is
        # [128, NE*128] with col (j*128+e) = WT[j*128+p, et*128+e]
        Wr = WT.reshape(NE, 128, NE, 128)          # [j, p, et, e]
        return np.ascontiguousarray(
            Wr.transpose(1, 2, 0, 3).reshape(128, NE * NE * 128))

    wq_p = pack_w((Wq.T * scale).astype(np.float32)).astype(ml_dtypes.bfloat16)
    wk_p = pack_w(Wk.T.astype(np.float32)).astype(ml_dtypes.bfloat16)
    # wo packed: stripe e is [128, E] with col eo = Wo.T[e*128+p, eo]
    wo_p = np.ascontiguousarray(
        Wo.T.reshape(NE, 128, E).transpose(1, 0, 2).reshape(128, NE * E)
    ).astype(ml_dtypes.bfloat16)
    bq_a = np.ascontiguousarray((bq * scale).reshape(NE, 128).T).astype(np.float32)
    bk_a = np.ascontiguousarray(bk.reshape(NE, 128).T).astype(np.float32)
    bo_a = np.tile(bo.astype(np.float32)[None, :], (128, 1))

    in_maps = []
    for c in range(N_CORES):
        heads = [HPC * c + h for h in range(HPC)]
        # Wv^T slice packed: [128, NE*HPC*DH], col block j -> Wv[e_g, j*128+p]
        wv_cols = np.concatenate(
            [Wv[ho * DH:(ho + 1) * DH, :] for ho in heads], axis=0)  # [128,E]
        wv_p = np.ascontiguousarray(
            wv_cols.T.reshape(NE, 128, HPC * DH).transpose(1, 0, 2)
            .reshape(128, NE * HPC * DH)).astype(ml_dtypes.bfloat16)
        bv_a = np.zeros((128, HPC), np.float32)
        for h, ho in enumerate(heads):
            bv_a[0:DH, h] = bv[ho * DH:(ho + 1) * DH]
        wvec = np.zeros((128, HPC * KS * NE), np.float32)
        for h, ho in enumerate(heads):
            for dk in range(KS):
                col_v = np.repeat(conv_w[ho, :, dk], DH)       # [E]
                for cc in range(NE):
                    wvec[:, (h * KS + dk) * NE + cc] = col_v[cc * 128:(cc + 1) * 128]
        in_maps.append({
            "qTh": qTh,
            "qSl": np.ascontiguousarray(qTh[:, c * QSLICE:(c + 1) * QSLICE]),
            "wq_p": wq_p, "wk_p": wk_p, "wv_p": wv_p,
            "wo_p": wo_p, "bq": bq_a, "bk": bk_a, "bv": bv_a,
            "bo": bo_a, "wvec": wvec,
        })
    return in_maps


_NC_CACHE = {}


def kernel(**inputs) -> np.ndarray:
    in_maps = prep_in_maps(**{k: np.asarray(v) for k, v in inputs.items()})
    if "nc" not in _NC_CACHE:
        _NC_CACHE["nc"] = build_nc()
    nc = _NC_CACHE["nc"]
    res = run_bass_kernel_spmd(nc, in_maps, list(range(N_CORES)))
    full = np.concatenate([res.results[c]["out"] for c in range(N_CORES)],
                          axis=0)
    return full.reshape(B, S, E).astype(np.float32)


# revision 16
# speedup vs baseline: 1.2194x; 1.1562x over previous
"""Trainium2 Bass kernel for ConvolutionalAttention (B=2,S=2048,E=1024,H=16,KS=3).

Reference:  Q,K,V = query @ W.T + b;  scores = QK^T/sqrt(Dh) per head;
cross-head conv1d (H->H channels, kernel 3) along the key axis; softmax over
keys; out = (weights @ V) merged heads @ Wo.T + bo.

Strategy (8 cores, head-parallel, conv folded into K, fp8 QK):
  K_conv[ho][k,(hi,d)] = sum_dk conv_w[ho,hi,dk] * K[k+dk-1,(hi,d)]
  => scores_conv[ho] = Q_full @ K_conv[ho]^T   (E=1024-deep matmul, computed
  transposed as [k,q]).  Each core owns H/8 = 2 output heads for all (b,q):
    1. each core computes only its OWN s-chunk (512 cols) of Q^T and K^T
       (stationary packed weight stripes), evacuates Q^T as fp8e4 and K^T
       as bf16 to DRAM, then AllGathers both across the 8 cores; V for the
       core's 2 heads is computed locally from query^T halves (the V
       matmuls fill the TensorE idle while the AllGathers fly);
    2. K_conv is formed on VectorE (tensor_scalar + 2 fused
       scalar_tensor_tensor AXPYs, bf16) + a ScalarE fp8 downcast into a
       RESIDENT fp8 SBUF tensor [128, HPC, NE, BS], in four (head, batch)
       passes ordered to unblock the first attention block earliest;
       Q^T fp8 is loaded into a resident [128, NE, BS] SBUF tensor.
       No DRAM traffic at all inside the attention loops.
    3. per (b, head): QK_conv as fp8 DoubleRow matmuls (256-deep per pass,
       [128,2,128] stationaries x [128,2,512] moving, 4 q-chunks per
       stationary) -> PSUM f32 -> Exp on ScalarE (bf16 out) -> PV matmuls
       against ones-augmented V so the softmax denominator lands in PSUM
       row 64 -> reciprocal -> K=1-matmul broadcast -> normalize (+bv,
       bf16 out).  bv is exact post-softmax (weights sum to 1); conv_b
       cancels inside softmax; 1/sqrt(Dh) folded into Wq/bq on host;
    4. AllToAll (bf16) reshards (head-slice -> q-slice); final Wo
       projection of this core's 512 output rows (bf16, f32 accum + bias).
"""
import numpy as np
import ml_dtypes

import concourse.bacc as bacc
import concourse.mybir as mybir
import concourse.tile as tile
from concourse.bass_utils import run_bass_kernel_spmd

B, S, E, H, KS = 2, 2048, 1024, 16, 3
DH = E // H                  # 64
N_CORES = 8
HPC = H // N_CORES           # 2 heads per core
BS = B * S                   # 4096
QSLICE = BS // N_CORES       # 512 output rows per core
NE = E // 128                # 8 contraction chunks
NEP = NE // 2                # 4 fp8 DoubleRow e-chunk pairs
NKT = S // 128               # 16 k-tiles per batch
NQC = S // 512               # 4 q-chunks per batch
VROW = DH + 1                # 65: head block in augmented V
KTC_W = S + 2                # per-batch K^T chunk: [z | b:S | z]

F32 = mybir.dt.float32
BF16 = mybir.dt.bfloat16
F8 = mybir.dt.float8e4
AL = mybir.AluOpType
AF = mybir.ActivationFunctionType
DR = mybir.MatmulPerfMode.DoubleRow


def build_nc(n_cores=N_CORES, collective=True):
    nc = bacc.Bacc("TRN2", target_bir_lowering=False, debug=False,
                   num_devices=n_cores)
    # inputs (host-prepped layouts; see prep_in_maps)
    qTh = nc.dram_tensor("qTh", [E, BS], BF16, kind="ExternalInput")
    wq_p = nc.dram_tensor("wq_p", [128, NE * NE * 128], BF16, kind="ExternalInput")
    wk_p = nc.dram_tensor("wk_p", [128, NE * NE * 128], BF16, kind="ExternalInput")
    wv_p = nc.dram_tensor("wv_p", [128, NE * HPC * DH], BF16, kind="ExternalInput")
    wo_p = nc.dram_tensor("wo_p", [128, NE * E], BF16, kind="ExternalInput")
    bq = nc.dram_tensor("bq", [128, NE], F32, kind="ExternalInput")
    bk = nc.dram_tensor("bk", [128, NE], F32, kind="ExternalInput")
    bv = nc.dram_tensor("bv", [128, HPC], F32, kind="ExternalInput")
    bo = nc.dram_tensor("bo", [128, E], F32, kind="ExternalInput")
    wvec = nc.dram_tensor("wvec", [128, HPC * KS * NE], F32, kind="ExternalInput")
    out = nc.dram_tensor("out", [QSLICE, E], F32, kind="ExternalOutput")
    # own s-chunk of query^T, supplied per-core (core c gets cols c*512..)
    qSl = nc.dram_tensor("qSl", [E, QSLICE], BF16, kind="ExternalInput")

    # queue mode: sequential pools get distinct SBUF addresses (ring) so a
    # released pool's addresses are not immediately reused — avoids false
    # cross-phase dependencies (formation pools vs attention pools)
    with tile.TileContext(nc, pool_alloc_mode="queue") as tc:
        with (
            tc.tile_pool(name="dram", bufs=1, space="DRAM") as dram,
            tc.tile_pool(name="persist", bufs=1) as persist,
        ):
            k_own = dram.tile([E, QSLICE], BF16)
            q_own = dram.tile([E, QSLICE], F8)
            k_all = dram.tile([N_CORES * E, QSLICE], BF16, addr_space="Shared")
            q_all = dram.tile([N_CORES * E, QSLICE], F8, addr_space="Shared")
            a2a_in = dram.tile([N_CORES * 128, QSLICE], BF16)
            a2a_out = dram.tile([N_CORES * 128, QSLICE], BF16)

            # resident attention operands
            kc_sb = persist.tile([128, HPC, NE, BS], F8)   # K_conv (fp8)
            q_sb = persist.tile([128, NE, BS], F8)         # Q^T (fp8)
            # augmented V: cols = g*(HPC*VROW) + h*VROW + [0..63]=d, 64=ones
            # where g = b*NKT + kt is the global k-tile index (32 of them)
            v_sb = persist.tile([128, B * NKT * HPC * VROW], BF16)
            bv_sb = persist.tile([128, HPC], F32)
            wvec_sb = persist.tile([128, HPC * KS * NE], F32)
            ones_sb = persist.tile([1, DH], BF16)
            wo_sb = persist.tile([128, NE * E], BF16)
            bo_sb = persist.tile([128, E], F32)
            nc.sync.dma_start(bv_sb[:], bv[:, :])
            nc.sync.dma_start(wvec_sb[:], wvec[:, :])
            nc.vector.memset(ones_sb[:], 1.0)
            for g in range(B * NKT):
                for h in range(HPC):
                    c0 = g * HPC * VROW + h * VROW + DH
                    nc.vector.memset(v_sb[:, c0:c0 + 1], 1.0)

            # -------- phase 1a: own-chunk Q/K projections + AllGathers -----
            with (
                tc.tile_pool(name="proj", bufs=1) as proj,
                tc.tile_pool(name="pw", bufs=NE) as pw,
                tc.tile_pool(name="pevac", bufs=3) as pevac,
                tc.tile_pool(name="ppsum", bufs=3, space="PSUM") as ppsum,
            ):
                qt_own = proj.tile([128, NE * QSLICE], BF16, tag="qtown")
                bq_sb = proj.tile([128, NE], F32, tag="bq")
                bk_sb = proj.tile([128, NE], F32, tag="bk")

                # own s-chunk of raw query^T first: it + the first K stripe
                # gate the K projection whose AllGather is the critical path
                for j in range(NE):
                    nc.sync.dma_start(
                        qt_own[:, j * QSLICE:(j + 1) * QSLICE],
                        qSl[j * 128:(j + 1) * 128, :])
                # prefetch ALL weight stripes upfront (K first): the K-chunk
                # evac writes must not queue behind trickling stripe DMAs,
                # since the K AllGather trigger is the critical path
                wk_stripes, wq_stripes = [], []
                for et in range(NE):
                    wk_sb = pw.tile([128, NE * 128], BF16, tag="wks",
                                    name=f"wkp{et}")
                    nc.sync.dma_start(wk_sb[:], wk_p[:, et * E:(et + 1) * E])
                    wk_stripes.append(wk_sb)
                nc.sync.dma_start(bk_sb[:], bk[:, :])
                for et in range(NE):
                    wq_sb = pw.tile([128, NE * 128], BF16, tag="wqs",
                                    name=f"wqp{et}")
                    nc.sync.dma_start(wq_sb[:], wq_p[:, et * E:(et + 1) * E])
                    wq_stripes.append(wq_sb)
                nc.sync.dma_start(bq_sb[:], bq[:, :])

                # K^T own chunk -> bf16 DRAM; Q^T own chunk -> fp8 DRAM
                for which in ("k", "q"):
                    for et in range(NE):
                        w_sb = (wq_stripes if which == "q" else wk_stripes)[et]
                        ps = ppsum.tile([128, QSLICE], F32, tag="pp")
                        for j in range(NE):
                            nc.tensor.matmul(
                                ps[:], w_sb[:, j * 128:(j + 1) * 128],
                                qt_own[:, j * QSLICE:(j + 1) * QSLICE],
                                start=(j == 0), stop=(j == NE - 1))
                        if which == "k":
                            ke = pevac.tile([128, QSLICE], BF16, tag="kevac")
                            nc.scalar.activation(ke[:], ps[:], AF.Identity,
                                                 bias=bk_sb[:, et:et + 1],
                                                 scale=1.0)
                            nc.sync.dma_start(
                                k_own[et * 128:(et + 1) * 128, :], ke[:])
                        else:
                            qe = pevac.tile([128, QSLICE], F8, tag="qevac")
                            nc.scalar.activation(qe[:], ps[:], AF.Identity,
                                                 bias=bq_sb[:, et:et + 1],
                                                 scale=1.0)
                            nc.sync.dma_start(
                                q_own[et * 128:(et + 1) * 128, :], qe[:])
                    if which == "k" and collective:
                        nc.gpsimd.collective_compute(
                            "AllGather", AL.bypass,
                            replica_groups=[list(range(n_cores))],
                            ins=[k_own.opt()], outs=[k_all.opt()])
                if collective:
                    nc.gpsimd.collective_compute(
                        "AllGather", AL.bypass,
                        replica_groups=[list(range(n_cores))],
                        ins=[q_own.opt()], outs=[q_all.opt()])

            # -------- phase 1b: V (own 2 heads) from query^T halves --------
            # fills the TensorE idle while the AllGathers run on TOPSP/SDMA
            with (
                tc.tile_pool(name="vproj", bufs=1) as vproj,
                tc.tile_pool(name="qth", bufs=2) as qth,
                tc.tile_pool(name="vpsum", bufs=2, space="PSUM") as vpsum,
            ):
                wv_sb = vproj.tile([128, NE * HPC * DH], BF16, tag="wv")
                nc.sync.dma_start(wv_sb[:], wv_p[:, :])
                QW = 1024                           # s-cols per quarter tile
                for qtr in range(BS // QW):
                    qh = qth.tile([128, NE * QW], BF16, tag="qth")
                    for j in range(NE):
                        nc.sync.dma_start(
                            qh[:, j * QW:(j + 1) * QW],
                            qTh[j * 128:(j + 1) * 128,
                                qtr * QW:(qtr + 1) * QW])
                    for gl in range(QW // 128):    # s-tile within quarter
                        g = qtr * (QW // 128) + gl
                        pv = vpsum.tile([128, HPC * DH], F32, tag="pv")
                        for j in range(NE):
                            nc.tensor.matmul(
                                pv[:], qh[:, j * QW + gl * 128:
                                          j * QW + (gl + 1) * 128],
                                wv_sb[:, j * HPC * DH:(j + 1) * HPC * DH],
                                start=(j == 0), stop=(j == NE - 1))
                        for h in range(HPC):
                            c0 = g * HPC * VROW + h * VROW
                            nc.scalar.activation(
                                v_sb[:, c0:c0 + DH],
                                pv[:, h * DH:(h + 1) * DH], AF.Copy)

            # -------- phase 2: form K_conv (fp8) + load Q^T fp8 --------
            with (
                tc.tile_pool(name="ktp", bufs=3) as ktp,
                tc.tile_pool(name="kcv", bufs=2) as kcv,
            ):
                # four (head, batch) passes, first-attention-block first;
                # each pass streams the 8 e-chunks of K^T for its batch
                for pi_, (h, b_i) in enumerate(((0, 0), (1, 0), (0, 1), (1, 1))):
                    for cc in range(NE):
                        ktc = ktp.tile([128, KTC_W], BF16, tag="ktc")
                        nc.vector.memset(ktc[:, 0:1], 0.0)
                        nc.vector.memset(ktc[:, S + 1:S + 2], 0.0)
                        for pos in range(NQC):
                            sc = b_i * NQC + pos   # source core for this span
                            nc.sync.dma_start(
                                ktc[:, 1 + pos * QSLICE:1 + (pos + 1) * QSLICE],
                                k_all[sc * E + cc * 128:
                                      sc * E + (cc + 1) * 128, :])

                        def wv_(dk):
                            col = (h * KS + dk) * NE + cc
                            return wvec_sb[:, col:col + 1]

                        # all on VectorE: ScalarE stays free for the EXPs
                        # (its FIFO queue would otherwise head-of-line block
                        # attention behind the downcasts)
                        t0 = kcv.tile([128, S], BF16, tag="t0")
                        t1 = kcv.tile([128, S], BF16, tag="t1")
                        nc.vector.tensor_scalar(
                            t0[:], ktc[:, 0:S], wv_(0), None, AL.mult)
                        nc.vector.tensor_scalar(
                            t1[:], ktc[:, 1:1 + S], wv_(1), None, AL.mult)
                        nc.vector.tensor_tensor(t0[:], t0[:], t1[:], AL.add)
                        nc.vector.tensor_scalar(
                            t1[:], ktc[:, 2:2 + S], wv_(2), None, AL.mult)
                        nc.vector.tensor_tensor(
                            kc_sb[:, h, cc, b_i * S:(b_i + 1) * S],
                            t0[:], t1[:], AL.add)
                    if pi_ == 1:
                        # Q^T fp8 resident load, after the first two K_conv
                        # passes' loads (readiness order: AG_K before AG_Q)
                        for j in range(NE):
                            for sc in range(N_CORES):
                                nc.sync.dma_start(
                                    q_sb[:, j, sc * QSLICE:(sc + 1) * QSLICE],
                                    q_all[sc * E + j * 128:
                                          sc * E + (j + 1) * 128, :])

            # ---------------- phase 3: attention ----------------
            with (
                tc.tile_pool(name="esb", bufs=6) as esb,
                tc.tile_pool(name="norm", bufs=4) as norm,
                tc.tile_pool(name="qkpsum", bufs=4, space="PSUM") as qkpsum,
                tc.tile_pool(name="pvpsum", bufs=4, space="PSUM") as pvpsum,
            ):
                for b_i in range(B):
                    for h in range(HPC):
                        pvs = [pvpsum.tile([VROW, 512], F32, tag="pvp",
                                           name=f"pv{qq}")
                               for qq in range(NQC)]
                        inv_sb = norm.tile([1, S], BF16, tag="inv")
                        for kt in range(NKT):
                            g = b_i * NKT + kt
                            c0 = g * HPC * VROW + h * VROW
                            pss = [qkpsum.tile([128, 512], F32, tag="qk",
                                               name=f"qk{qc}")
                                   for qc in range(NQC)]
                            for c in range(NEP):
                                lhsT = kc_sb[:, h, 2 * c:2 * c + 2,
                                             b_i * S + kt * 128:
                                             b_i * S + (kt + 1) * 128]
                                for qc in range(NQC):
                                    rhs = q_sb[:, 2 * c:2 * c + 2,
                                               b_i * S + qc * 512:
                                               b_i * S + (qc + 1) * 512]
                                    nc.tensor.matmul(
                                        pss[qc][:], lhsT, rhs,
                                        start=(c == 0), stop=(c == NEP - 1),
                                        perf_mode=DR)
                            for qc in range(NQC):
                                ex = esb.tile([128, 512], BF16, tag="exp")
                                nc.scalar.activation(ex[:], pss[qc][:], AF.Exp)
                                nc.tensor.matmul(
                                    pvs[qc][:], v_sb[:, c0:c0 + VROW],
                                    ex[:], start=(kt == 0),
                                    stop=(kt == NKT - 1))
                        # normalize + bias, ship to a2a bounce
                        for qc in range(NQC):
                            with nc.allow_low_precision(
                                    reason="softmax denom bf16 bcast"):
                                nc.vector.reciprocal(
                                    inv_sb[0:1, qc * 512:(qc + 1) * 512],
                                    pvs[qc][DH:DH + 1, :])
                            # shares the qk bank ring (4 qk + 4 pv = 8 banks)
                            pi = qkpsum.tile([DH, 512], F32, tag="qk",
                                             name="pi")
                            nc.tensor.matmul(
                                pi[:], ones_sb[0:1, :],
                                inv_sb[0:1, qc * 512:(qc + 1) * 512],
                                start=True, stop=True)
                            ib = norm.tile([DH, 512], F32, tag="invbc")
                            nc.scalar.activation(ib[:], pi[:], AF.Copy)
                            ho = norm.tile([DH, 512], BF16, tag="ho")
                            nc.vector.tensor_tensor(ho[:], pvs[qc][0:DH, :],
                                                    ib[:], AL.mult)
                            nc.vector.tensor_scalar(
                                ho[:], ho[:], bv_sb[0:DH, h:h + 1], None, AL.add)
                            piece = b_i * NQC + qc
                            r0 = piece * 128 + h * DH
                            nc.sync.dma_start(a2a_in[r0:r0 + DH, :], ho[:])

            # output-projection weights: loaded late so the bulk DMAs do
            # not delay the phase-1 critical path
            nc.sync.dma_start(wo_sb[:], wo_p[:, :])
            nc.sync.dma_start(bo_sb[:], bo[:, :])

            # ---------------- phase 4: exchange + output proj ----------------
            if collective:
                nc.gpsimd.collective_compute(
                    "AllToAll", AL.bypass,
                    replica_groups=[list(range(n_cores))],
                    ins=[a2a_in.opt()], outs=[a2a_out.opt()])
            else:
                nc.sync.dma_start(a2a_out[:, :], a2a_in[:, :])

            with (
                tc.tile_pool(name="fin", bufs=1) as fin,
                tc.tile_pool(name="fevac", bufs=3) as fevac,
                tc.tile_pool(name="fpsum", bufs=2, space="PSUM") as fpsum,
            ):
                go_sb = fin.tile([128, NE * QSLICE], BF16, tag="go")
                for e in range(NE):
                    nc.sync.dma_start(go_sb[:, e * QSLICE:(e + 1) * QSLICE],
                                      a2a_out[e * 128:(e + 1) * 128, :])
                NH = E // 512
                for qt in range(QSLICE // 128):
                    pfs = [fpsum.tile([128, 512], F32, tag="pf",
                                      name=f"pf{nh}") for nh in range(NH)]
                    for e in range(NE):
                        lhsT = go_sb[:, e * QSLICE + qt * 128:
                                     e * QSLICE + (qt + 1) * 128]
                        for nh in range(NH):
                            rhs = wo_sb[:, e * E + nh * 512:
                                        e * E + (nh + 1) * 512]
                            nc.tensor.matmul(pfs[nh][:], lhsT, rhs,
                                             start=(e == 0),
                                             stop=(e == NE - 1))
                    for nh in range(NH):
                        ot = fevac.tile([128, 512], F32, tag="ot")
                        nc.vector.tensor_tensor(
                            ot[:], pfs[nh][:], bo_sb[:, nh * 512:(nh + 1) * 512],
                            AL.add)
                        nc.sync.dma_start(
                            out[qt * 128:(qt + 1) * 128,
                                nh * 512:(nh + 1) * 512], ot[:])
    nc.compile()
    return nc


def prep_in_maps(query, Wq, bq, Wk, bk, Wv, bv, Wo, bo, conv_w, conv_b):
    """Host-side layout prep. conv_b is dropped: softmax(x+c) == softmax(x)."""
    del conv_b
    scale = 1.0 / np.sqrt(DH)
    qT = np.ascontiguousarray(query.reshape(BS, E).T)           # [E, BS]
    qTh = qT.astype(ml_dtypes.bfloat16)

    def pack_w(WT):  # [E_in, E_out] -> [128, NE*NE*128], stripe et is
        # [128, NE*128] with col (j*128+e) = WT[j*128+p, et*128+e]
        Wr = WT.reshape(NE, 128, NE, 128)          # [j, p, et, e]
        return np.ascontiguousarray(
            Wr.transpose(1, 2, 0, 3).reshape(128, NE * NE * 128))

    wq_p = pack_w((Wq.T * scale).astype(np.float32)).astype(ml_dtypes.bfloat16)
    wk_p = pack_w(Wk.T.astype(np.float32)).astype(ml_dtypes.bfloat16)
    # wo packed: stripe e is [128, E] with col eo = Wo.T[e*128+p, eo]
    wo_p = np.ascontiguousarray(
        Wo.T.reshape(NE, 128, E).transpose(1, 0, 2).reshape(128, NE * E)
    ).astype(ml_dtypes.bfloat16)
    bq_a = np.ascontiguousarray((bq * scale).reshape(NE, 128).T).astype(np.float32)
    bk_a = np.ascontiguousarray(bk.reshape(NE, 128).T).astype(np.float32)
    bo_a = np.tile(bo.astype(np.float32)[None, :], (128, 1))

    in_maps = []
    for c in range(N_CORES):
        heads = [HPC * c + h for h in range(HPC)]
        # Wv^T slice packed: [128, NE*HPC*DH], col block j -> Wv[e_g, j*128+p]
        wv_cols = np.concatenate(
            [Wv[ho * DH:(ho + 1) * DH, :] for ho in heads], axis=0)  # [128,E]
        wv_p = np.ascontiguousarray(
            wv_cols.T.reshape(NE, 128, HPC * DH).transpose(1, 0, 2)
            .reshape(128, NE * HPC * DH)).astype(ml_dtypes.bfloat16)
        bv_a = np.zeros((128, HPC), np.float32)
        for h, ho in enumerate(heads):
            bv_a[0:DH, h] = bv[ho * DH:(ho + 1) * DH]
        wvec = np.zeros((128, HPC * KS * NE), np.float32)
        for h, ho in enumerate(heads):
            for dk in range(KS):
                col_v = np.repeat(conv_w[ho, :, dk], DH)       # [E]
                for cc in range(NE):
                    wvec[:, (h * KS + dk) * NE + cc] = col_v[cc * 128:(cc + 1) * 128]
        in_maps.append({
            "qTh": qTh,
            "qSl": np.ascontiguousarray(qTh[:, c * QSLICE:(c + 1) * QSLICE]),
            "wq_p": wq_p, "wk_p": wk_p, "wv_p": wv_p,
            "wo_p": wo_p, "bq": bq_a, "bk": bk_a, "bv": bv_a,
            "bo": bo_a, "wvec": wvec,
        })
    return in_maps


_NC_CACHE = {}


def kernel(**inputs) -> np.ndarray:
    in_maps = prep_in_maps(**{k: np.asarray(v) for k, v in inputs.items()})
    if "nc" not in _NC_CACHE:
        _NC_CACHE["nc"] = build_nc()
    nc = _NC_CACHE["nc"]
    res = run_bass_kernel_spmd(nc, in_maps, list(range(N_CORES)))
    full = np.concatenate([res.results[c]["out"] for c in range(N_CORES)],
                          axis=0)
    return full.reshape(B, S, E).astype(np.float32)


# revision 17
# speedup vs baseline: 1.2816x; 1.0510x over previous
"""Trainium2 Bass kernel for ConvolutionalAttention (B=2,S=2048,E=1024,H=16,KS=3).

Reference:  Q,K,V = query @ W.T + b;  scores = QK^T/sqrt(Dh) per head;
cross-head conv1d (H->H channels, kernel 3) along the key axis; softmax over
keys; out = (weights @ V) merged heads @ Wo.T + bo.

Strategy (8 cores, head-parallel, conv folded into K, fp8 QK):
  K_conv[ho][k,(hi,d)] = sum_dk conv_w[ho,hi,dk] * K[k+dk-1,(hi,d)]
  => scores_conv[ho] = Q_full @ K_conv[ho]^T   (E=1024-deep matmul, computed
  transposed as [k,q]).  Each core owns H/8 = 2 output heads for all (b,q):
    1. each core computes only its OWN s-chunk (512 cols) of Q^T and K^T
       (stationary packed weight stripes), evacuates Q^T as fp8e4 and K^T
       as bf16 to DRAM, then AllGathers both across the 8 cores; V for the
       core's 2 heads is computed locally from query^T halves (the V
       matmuls fill the TensorE idle while the AllGathers fly);
    2. K_conv is formed on VectorE (tensor_scalar + 2 fused
       scalar_tensor_tensor AXPYs, bf16) + a ScalarE fp8 downcast into a
       RESIDENT fp8 SBUF tensor [128, HPC, NE, BS], in four (head, batch)
       passes ordered to unblock the first attention block earliest;
       Q^T fp8 is loaded into a resident [128, NE, BS] SBUF tensor.
       No DRAM traffic at all inside the attention loops.
    3. per (b, head): QK_conv as fp8 DoubleRow matmuls (256-deep per pass,
       [128,2,128] stationaries x [128,2,512] moving, 4 q-chunks per
       stationary) -> PSUM f32 -> Exp on ScalarE (bf16 out) -> PV matmuls
       against ones-augmented V so the softmax denominator lands in PSUM
       row 64 -> reciprocal -> K=1-matmul broadcast -> normalize (+bv,
       bf16 out).  bv is exact post-softmax (weights sum to 1); conv_b
       cancels inside softmax; 1/sqrt(Dh) folded into Wq/bq on host;
    4. AllToAll (bf16) reshards (head-slice -> q-slice); final Wo
       projection of this core's 512 output rows (bf16, f32 accum + bias).
"""
import numpy as np
import ml_dtypes

import concourse.bacc as bacc
import concourse.mybir as mybir
import concourse.tile as tile
from concourse.bass_utils import run_bass_kernel_spmd

B, S, E, H, KS = 2, 2048, 1024, 16, 3
DH = E // H                  # 64
N_CORES = 8
HPC = H // N_CORES           # 2 heads per core
BS = B * S                   # 4096
QSLICE = BS // N_CORES       # 512 output rows per core
NE = E // 128                # 8 contraction chunks
NEP = NE // 2                # 4 fp8 DoubleRow e-chunk pairs
NKT = S // 128               # 16 k-tiles per batch
NQC = S // 512               # 4 q-chunks per batch
VROW = DH + 1                # 65: head block in augmented V
KTC_W = S + 2                # per-batch K^T chunk: [z | b:S | z]

F32 = mybir.dt.float32
BF16 = mybir.dt.bfloat16
F8 = mybir.dt.float8e4
AL = mybir.AluOpType
AF = mybir.ActivationFunctionType
DR = mybir.MatmulPerfMode.DoubleRow


def build_nc(n_cores=N_CORES, collective=True):
    nc = bacc.Bacc("TRN2", target_bir_lowering=False, debug=False,
                   num_devices=n_cores)
    # inputs (host-prepped layouts; see prep_in_maps)
    qTh = nc.dram_tensor("qTh", [E, BS], BF16, kind="ExternalInput")
    wq_p = nc.dram_tensor("wq_p", [128, NE * NE * 128], BF16, kind="ExternalInput")
    wk_p = nc.dram_tensor("wk_p", [128, NE * NE * 128], BF16, kind="ExternalInput")
    wv_p = nc.dram_tensor("wv_p", [128, NE * HPC * DH], BF16, kind="ExternalInput")
    wo_p = nc.dram_tensor("wo_p", [128, NE * E], BF16, kind="ExternalInput")
    bq = nc.dram_tensor("bq", [128, NE], F32, kind="ExternalInput")
    bk = nc.dram_tensor("bk", [128, NE], F32, kind="ExternalInput")
    bv = nc.dram_tensor("bv", [128, HPC], F32, kind="ExternalInput")
    bo = nc.dram_tensor("bo", [128, E], F32, kind="ExternalInput")
    wvec = nc.dram_tensor("wvec", [128, HPC * KS * NE], F32, kind="ExternalInput")
    out = nc.dram_tensor("out", [QSLICE, E], F32, kind="ExternalOutput")
    # own s-chunk of query^T, supplied per-core (core c gets cols c*512..)
    qSl = nc.dram_tensor("qSl", [E, QSLICE], BF16, kind="ExternalInput")

    # queue mode: sequential pools get distinct SBUF addresses (ring) so a
    # released pool's addresses are not immediately reused — avoids false
    # cross-phase dependencies (formation pools vs attention pools)
    with tile.TileContext(nc, pool_alloc_mode="queue") as tc:
        with (
            tc.tile_pool(name="dram", bufs=1, space="DRAM") as dram,
            tc.tile_pool(name="persist", bufs=1) as persist,
        ):
            k_own = dram.tile([E, QSLICE], BF16)
            q_own = dram.tile([E, QSLICE], F8)
            k_all = dram.tile([N_CORES * E, QSLICE], BF16, addr_space="Shared")
            q_all = dram.tile([N_CORES * E, QSLICE], F8, addr_space="Shared")
            a2a_in = dram.tile([N_CORES * 128, QSLICE], BF16)
            a2a_out = dram.tile([N_CORES * 128, QSLICE], BF16)

            # resident attention operands
            kc_sb = persist.tile([128, HPC, NE, BS], F8)   # K_conv (fp8)
            q_sb = persist.tile([128, NE, BS], F8)         # Q^T (fp8)
            # augmented V: cols = g*(HPC*VROW) + h*VROW + [0..63]=d, 64=ones
            # where g = b*NKT + kt is the global k-tile index (32 of them)
            v_sb = persist.tile([128, B * NKT * HPC * VROW], BF16)
            bv_sb = persist.tile([128, HPC], F32)
            wvec_sb = persist.tile([128, HPC * KS * NE], F32)
            ones_sb = persist.tile([1, DH], BF16)
            wo_sb = persist.tile([128, NE * E], BF16)
            bo_sb = persist.tile([128, E], F32)
            nc.sync.dma_start(bv_sb[:], bv[:, :])
            nc.sync.dma_start(wvec_sb[:], wvec[:, :])
            nc.vector.memset(ones_sb[:], 1.0)
            for g in range(B * NKT):
                for h in range(HPC):
                    c0 = g * HPC * VROW + h * VROW + DH
                    nc.vector.memset(v_sb[:, c0:c0 + 1], 1.0)

            # -------- phase 1a: own-chunk Q/K projections + AllGathers -----
            with (
                tc.tile_pool(name="proj", bufs=1) as proj,
                tc.tile_pool(name="pw", bufs=NE) as pw,
                tc.tile_pool(name="pevac", bufs=3) as pevac,
                tc.tile_pool(name="ppsum", bufs=3, space="PSUM") as ppsum,
            ):
                qt_own = proj.tile([128, NE * QSLICE], BF16, tag="qtown")
                bq_sb = proj.tile([128, NE], F32, tag="bq")
                bk_sb = proj.tile([128, NE], F32, tag="bk")

                # own s-chunk of raw query^T first: it + the first K stripe
                # gate the K projection whose AllGather is the critical path
                for j in range(NE):
                    nc.sync.dma_start(
                        qt_own[:, j * QSLICE:(j + 1) * QSLICE],
                        qSl[j * 128:(j + 1) * 128, :])
                # prefetch ALL weight stripes upfront (K first): the K-chunk
                # evac writes must not queue behind trickling stripe DMAs,
                # since the K AllGather trigger is the critical path
                wk_stripes, wq_stripes = [], []
                for et in range(NE):
                    wk_sb = pw.tile([128, NE * 128], BF16, tag="wks",
                                    name=f"wkp{et}")
                    nc.sync.dma_start(wk_sb[:], wk_p[:, et * E:(et + 1) * E])
                    wk_stripes.append(wk_sb)
                nc.sync.dma_start(bk_sb[:], bk[:, :])
                for et in range(NE):
                    wq_sb = pw.tile([128, NE * 128], BF16, tag="wqs",
                                    name=f"wqp{et}")
                    nc.sync.dma_start(wq_sb[:], wq_p[:, et * E:(et + 1) * E])
                    wq_stripes.append(wq_sb)
                nc.sync.dma_start(bq_sb[:], bq[:, :])

                # K^T own chunk -> bf16 DRAM; Q^T own chunk -> fp8 DRAM
                for which in ("k", "q"):
                    for et in range(NE):
                        w_sb = (wq_stripes if which == "q" else wk_stripes)[et]
                        ps = ppsum.tile([128, QSLICE], F32, tag="pp")
                        for j in range(NE):
                            nc.tensor.matmul(
                                ps[:], w_sb[:, j * 128:(j + 1) * 128],
                                qt_own[:, j * QSLICE:(j + 1) * QSLICE],
                                start=(j == 0), stop=(j == NE - 1))
                        if which == "k":
                            ke = pevac.tile([128, QSLICE], BF16, tag="kevac")
                            nc.scalar.activation(ke[:], ps[:], AF.Identity,
                                                 bias=bk_sb[:, et:et + 1],
                                                 scale=1.0)
                            nc.sync.dma_start(
                                k_own[et * 128:(et + 1) * 128, :], ke[:])
                        else:
                            qe = pevac.tile([128, QSLICE], F8, tag="qevac")
                            nc.scalar.activation(qe[:], ps[:], AF.Identity,
                                                 bias=bq_sb[:, et:et + 1],
                                                 scale=1.0)
                            nc.sync.dma_start(
                                q_own[et * 128:(et + 1) * 128, :], qe[:])
                    if which == "k" and collective:
                        nc.gpsimd.collective_compute(
                            "AllGather", AL.bypass,
                            replica_groups=[list(range(n_cores))],
                            ins=[k_own.opt()], outs=[k_all.opt()])
                if collective:
                    nc.gpsimd.collective_compute(
                        "AllGather", AL.bypass,
                        replica_groups=[list(range(n_cores))],
                        ins=[q_own.opt()], outs=[q_all.opt()])

            # -------- phase 1b: V (own 2 heads) from query^T halves --------
            # fills the TensorE idle while the AllGathers run on TOPSP/SDMA
            with (
                tc.tile_pool(name="vproj", bufs=1) as vproj,
                tc.tile_pool(name="qth", bufs=2) as qth,
                tc.tile_pool(name="vpsum", bufs=2, space="PSUM") as vpsum,
            ):
                wv_sb = vproj.tile([128, NE * HPC * DH], BF16, tag="wv")
                nc.sync.dma_start(wv_sb[:], wv_p[:, :])
                QW = 1024                           # s-cols per quarter tile
                for qtr in range(BS // QW):
                    qh = qth.tile([128, NE * QW], BF16, tag="qth")
                    for j in range(NE):
                        nc.sync.dma_start(
                            qh[:, j * QW:(j + 1) * QW],
                            qTh[j * 128:(j + 1) * 128,
                                qtr * QW:(qtr + 1) * QW])
                    for gl in range(QW // 128):    # s-tile within quarter
                        g = qtr * (QW // 128) + gl
                        pv = vpsum.tile([128, HPC * DH], F32, tag="pv")
                        for j in range(NE):
                            nc.tensor.matmul(
                                pv[:], qh[:, j * QW + gl * 128:
                                          j * QW + (gl + 1) * 128],
                                wv_sb[:, j * HPC * DH:(j + 1) * HPC * DH],
                                start=(j == 0), stop=(j == NE - 1))
                        for h in range(HPC):
                            c0 = g * HPC * VROW + h * VROW
                            nc.scalar.activation(
                                v_sb[:, c0:c0 + DH],
                                pv[:, h * DH:(h + 1) * DH], AF.Copy)

            # -------- phases 2+3 interleaved: K_conv passes + attention ----
            # formation pass for block i+1 is emitted between attention
            # blocks so the VectorE FIFO interleaves formation with the
            # normalize ops instead of queueing all formation first
            with (
                tc.tile_pool(name="ktp", bufs=3) as ktp,
                tc.tile_pool(name="kcv", bufs=2) as kcv,
                tc.tile_pool(name="esb", bufs=6) as esb,
                tc.tile_pool(name="norm", bufs=4) as norm,
                tc.tile_pool(name="qkpsum", bufs=4, space="PSUM") as qkpsum,
                tc.tile_pool(name="pvpsum", bufs=4, space="PSUM") as pvpsum,
            ):
                def form_pass(h, b_i):
                    for cc in range(NE):
                        ktc = ktp.tile([128, KTC_W], BF16, tag="ktc")
                        nc.vector.memset(ktc[:, 0:1], 0.0)
                        nc.vector.memset(ktc[:, S + 1:S + 2], 0.0)
                        for pos in range(NQC):
                            sc = b_i * NQC + pos
                            nc.sync.dma_start(
                                ktc[:, 1 + pos * QSLICE:1 + (pos + 1) * QSLICE],
                                k_all[sc * E + cc * 128:
                                      sc * E + (cc + 1) * 128, :])

                        def wv_(dk):
                            col = (h * KS + dk) * NE + cc
                            return wvec_sb[:, col:col + 1]

                        t0 = kcv.tile([128, S], BF16, tag="t0")
                        t1 = kcv.tile([128, S], BF16, tag="t1")
                        nc.vector.tensor_scalar(
                            t0[:], ktc[:, 0:S], wv_(0), None, AL.mult)
                        nc.vector.tensor_scalar(
                            t1[:], ktc[:, 1:1 + S], wv_(1), None, AL.mult)
                        nc.vector.tensor_tensor(t0[:], t0[:], t1[:], AL.add)
                        nc.vector.tensor_scalar(
                            t1[:], ktc[:, 2:2 + S], wv_(2), None, AL.mult)
                        nc.vector.tensor_tensor(
                            kc_sb[:, h, cc, b_i * S:(b_i + 1) * S],
                            t0[:], t1[:], AL.add)

                def load_q_sb():
                    for j in range(NE):
                        for sc in range(N_CORES):
                            nc.sync.dma_start(
                                q_sb[:, j, sc * QSLICE:(sc + 1) * QSLICE],
                                q_all[sc * E + j * 128:
                                      sc * E + (j + 1) * 128, :])

                def attn_block(b_i, h):
                    pvs = [pvpsum.tile([VROW, 512], F32, tag="pvp",
                                       name=f"pv{qq}")
                           for qq in range(NQC)]
                    inv_sb = norm.tile([1, S], BF16, tag="inv")
                    for kt in range(NKT):
                        g = b_i * NKT + kt
                        c0 = g * HPC * VROW + h * VROW
                        pss = [qkpsum.tile([128, 512], F32, tag="qk",
                                           name=f"qk{qc}")
                               for qc in range(NQC)]
                        for c in range(NEP):
                            lhsT = kc_sb[:, h, 2 * c:2 * c + 2,
                                         b_i * S + kt * 128:
                                         b_i * S + (kt + 1) * 128]
                            for qc in range(NQC):
                                rhs = q_sb[:, 2 * c:2 * c + 2,
                                           b_i * S + qc * 512:
                                           b_i * S + (qc + 1) * 512]
                                nc.tensor.matmul(
                                    pss[qc][:], lhsT, rhs,
                                    start=(c == 0), stop=(c == NEP - 1),
                                    perf_mode=DR)
                        for qc in range(NQC):
                            ex = esb.tile([128, 512], BF16, tag="exp")
                            nc.scalar.activation(ex[:], pss[qc][:], AF.Exp)
                            nc.tensor.matmul(
                                pvs[qc][:], v_sb[:, c0:c0 + VROW],
                                ex[:], start=(kt == 0),
                                stop=(kt == NKT - 1))
                    # normalize + bias, ship to a2a bounce
                    for qc in range(NQC):
                        with nc.allow_low_precision(
                                reason="softmax denom bf16 bcast"):
                            nc.vector.reciprocal(
                                inv_sb[0:1, qc * 512:(qc + 1) * 512],
                                pvs[qc][DH:DH + 1, :])
                        # shares the qk bank ring (4 qk + 4 pv = 8 banks)
                        pi = qkpsum.tile([DH, 512], F32, tag="qk",
                                         name="pi")
                        nc.tensor.matmul(
                            pi[:], ones_sb[0:1, :],
                            inv_sb[0:1, qc * 512:(qc + 1) * 512],
                            start=True, stop=True)
                        ib = norm.tile([DH, 512], F32, tag="invbc")
                        nc.scalar.activation(ib[:], pi[:], AF.Copy)
                        ho = norm.tile([DH, 512], BF16, tag="ho")
                        nc.vector.tensor_tensor(ho[:], pvs[qc][0:DH, :],
                                                ib[:], AL.mult)
                        nc.vector.tensor_scalar(
                            ho[:], ho[:], bv_sb[0:DH, h:h + 1], None, AL.add)
                        piece = b_i * NQC + qc
                        r0 = piece * 128 + h * DH
                        nc.sync.dma_start(a2a_in[r0:r0 + DH, :], ho[:])

                form_pass(0, 0)
                form_pass(1, 0)
                load_q_sb()
                attn_block(0, 0)
                form_pass(0, 1)
                attn_block(0, 1)
                form_pass(1, 1)
                attn_block(1, 0)
                attn_block(1, 1)

            # output-projection weights: loaded late so the bulk DMAs do
            # not delay the phase-1 critical path
            nc.sync.dma_start(wo_sb[:], wo_p[:, :])
            nc.sync.dma_start(bo_sb[:], bo[:, :])

            # ---------------- phase 4: exchange + output proj ----------------
            if collective:
                nc.gpsimd.collective_compute(
                    "AllToAll", AL.bypass,
                    replica_groups=[list(range(n_cores))],
                    ins=[a2a_in.opt()], outs=[a2a_out.opt()])
            else:
                nc.sync.dma_start(a2a_out[:, :], a2a_in[:, :])

            with (
                tc.tile_pool(name="fin", bufs=1) as fin,
                tc.tile_pool(name="fevac", bufs=3) as fevac,
                tc.tile_pool(name="fpsum", bufs=2, space="PSUM") as fpsum,
            ):
                go_sb = fin.tile([128, NE * QSLICE], BF16, tag="go")
                for e in range(NE):
                    nc.sync.dma_start(go_sb[:, e * QSLICE:(e + 1) * QSLICE],
                                      a2a_out[e * 128:(e + 1) * 128, :])
                NH = E // 512
                for qt in range(QSLICE // 128):
                    pfs = [fpsum.tile([128, 512], F32, tag="pf",
                                      name=f"pf{nh}") for nh in range(NH)]
                    for e in range(NE):
                        lhsT = go_sb[:, e * QSLICE + qt * 128:
                                     e * QSLICE + (qt + 1) * 128]
                        for nh in range(NH):
                            rhs = wo_sb[:, e * E + nh * 512:
                                        e * E + (nh + 1) * 512]
                            nc.tensor.matmul(pfs[nh][:], lhsT, rhs,
                                             start=(e == 0),
                                             stop=(e == NE - 1))
                    for nh in range(NH):
                        ot = fevac.tile([128, 512], F32, tag="ot")
                        nc.vector.tensor_tensor(
                            ot[:], pfs[nh][:], bo_sb[:, nh * 512:(nh + 1) * 512],
                            AL.add)
                        nc.sync.dma_start(
                            out[qt * 128:(qt + 1) * 128,
                                nh * 512:(nh + 1) * 512], ot[:])
    nc.compile()
    return nc


def prep_in_maps(query, Wq, bq, Wk, bk, Wv, bv, Wo, bo, conv_w, conv_b):
    """Host-side layout prep. conv_b is dropped: softmax(x+c) == softmax(x)."""
    del conv_b
    scale = 1.0 / np.sqrt(DH)
    qT = np.ascontiguousarray(query.reshape(BS, E).T)           # [E, BS]
    qTh = qT.astype(ml_dtypes.bfloat16)

    def pack_w(WT):  # [E_in, E_out] -> [128, NE*NE*128], stripe et is
        # [128, NE*128] with col (j*128+e) = WT[j*128+p, et*128+e]
        Wr = WT.reshape(NE, 128, NE, 128)          # [j, p, et, e]
        return np.ascontiguousarray(
            Wr.transpose(1, 2, 0, 3).reshape(128, NE * NE * 128))

    wq_p = pack_w((Wq.T * scale).astype(np.float32)).astype(ml_dtypes.bfloat16)
    wk_p = pack_w(Wk.T.astype(np.float32)).astype(ml_dtypes.bfloat16)
    # wo packed: stripe e is [128, E] with col eo = Wo.T[e*128+p, eo]
    wo_p = np.ascontiguousarray(
        Wo.T.reshape(NE, 128, E).transpose(1, 0, 2).reshape(128, NE * E)
    ).astype(ml_dtypes.bfloat16)
    bq_a = np.ascontiguousarray((bq * scale).reshape(NE, 128).T).astype(np.float32)
    bk_a = np.ascontiguousarray(bk.reshape(NE, 128).T).astype(np.float32)
    bo_a = np.tile(bo.astype(np.float32)[None, :], (128, 1))

    in_maps = []
    for c in range(N_CORES):
        heads = [HPC * c + h for h in range(HPC)]
        # Wv^T slice packed: [128, NE*HPC*DH], col block j -> Wv[e_g, j*128+p]
        wv_cols = np.concatenate(
            [Wv[ho * DH:(ho + 1) * DH, :] for ho in heads], axis=0)  # [128,E]
        wv_p = np.ascontiguousarray(
            wv_cols.T.reshape(NE, 128, HPC * DH).transpose(1, 0, 2)
            .reshape(128, NE * HPC * DH)).astype(ml_dtypes.bfloat16)
        bv_a = np.zeros((128, HPC), np.float32)
        for h, ho in enumerate(heads):
            bv_a[0:DH, h] = bv[ho * DH:(ho + 1) * DH]
        wvec = np.zeros((128, HPC * KS * NE), np.float32)
        for h, ho in enumerate(heads):
            for dk in range(KS):
                col_v = np.repeat(conv_w[ho, :, dk], DH)       # [E]
                for cc in range(NE):
                    wvec[:, (h * KS + dk) * NE + cc] = col_v[cc * 128:(cc + 1) * 128]
        in_maps.append({
            "qTh": qTh,
            "qSl": np.ascontiguousarray(qTh[:, c * QSLICE:(c + 1) * QSLICE]),
            "wq_p": wq_p, "wk_p": wk_p, "wv_p": wv_p,
            "wo_p": wo_p, "bq": bq_a, "bk": bk_a, "bv": bv_a,
            "bo": bo_a, "wvec": wvec,
        })
    return in_maps


_NC_CACHE = {}


def kernel(**inputs) -> np.ndarray:
    in_maps = prep_in_maps(**{k: np.asarray(v) for k, v in inputs.items()})
    if "nc" not in _NC_CACHE:
        _NC_CACHE["nc"] = build_nc()
    nc = _NC_CACHE["nc"]
    res = run_bass_kernel_spmd(nc, in_maps, list(range(N_CORES)))
    full = np.concatenate([res.results[c]["out"] for c in range(N_CORES)],
                          axis=0)
    return full.reshape(B, S, E).astype(np.float32)
